# revision 1
# baseline (speedup 1.0000x reference)
"""Trainium2 Bass kernel for nn_ARFastWeightSwiGLU.

Auto-regressive fast-weight SwiGLU: chunked recurrence over 4 chunks.
Per chunk: a rank-MB weight update (forward + backward-style products on a
512-token mini-batch, per-row renormalization) followed by a SwiGLU forward
over the chunk's 1024 query tokens with the updated weights.

Sharding: pure data/head parallel over the merged batch*head dim b (16),
2 b-slices per NeuronCore, 8 cores, no collectives.

Layout strategy (per b):
  - master weights fp32 natural layout in SBUF, updated + renormalized there
  - bf16 transposed copies (regenerated per chunk via PE transpose) feed all
    matmuls; activations bf16; PSUM accumulation fp32
  - k/v/q arrive pre-cast to bf16 from the host; transposed operand tiles are
    loaded straight from HBM with the XBAR DMA transpose (bf16-only path)
  - lr tensors arrive pre-scaled by w_scale, so the update products need no
    extra scaling pass
"""

import sys

sys.path.insert(0, "/opt/trn_rl_repo")

from contextlib import ExitStack

import ml_dtypes
import numpy as np

import concourse.bacc as bacc
import concourse.bass as bass
import concourse.mybir as mybir
import concourse.tile as tile
from concourse.bass_utils import run_bass_kernel_spmd
from concourse.masks import make_identity

# Problem constants (hardcoded; kernel.py must be self-contained)
B, L, D, DH = 16, 4608, 256, 512
MB, UE = 512, 1024
W_SCALE = 0.01
FIRST = UE - MB          # 512
NCH = (L - FIRST) // UE  # 4
NCORES = 8
BPC = B // NCORES        # 2 b-slices per core
P = 128
MBT = MB // P            # 4 token tiles per mini-batch
DHT = DH // P            # 4 partition tiles of the hidden dim
DT = D // P              # 2 partition tiles of the head dim

F32 = mybir.dt.float32
BF16 = mybir.dt.bfloat16
AF = mybir.ActivationFunctionType

BF_NP = ml_dtypes.bfloat16


def build_program():
    nc = bacc.Bacc("TRN2", target_bir_lowering=False, debug=False)

    w0_in = nc.dram_tensor("w0_in", [BPC, DH, D], F32, kind="ExternalInput").ap()
    w1_in = nc.dram_tensor("w1_in", [BPC, D, DH], F32, kind="ExternalInput").ap()
    w2_in = nc.dram_tensor("w2_in", [BPC, DH, D], F32, kind="ExternalInput").ap()
    qb = nc.dram_tensor("qb", [BPC, L, D], BF16, kind="ExternalInput").ap()
    kb = nc.dram_tensor("kb", [BPC, NCH, MB, D], BF16, kind="ExternalInput").ap()
    vb = nc.dram_tensor("vb", [BPC, NCH, MB, D], BF16, kind="ExternalInput").ap()
    l0 = nc.dram_tensor("l0", [BPC, NCH, MB, D], BF16, kind="ExternalInput").ap()
    l1 = nc.dram_tensor("l1", [BPC, NCH, MB, DH], BF16, kind="ExternalInput").ap()
    l2 = nc.dram_tensor("l2", [BPC, NCH, MB, D], BF16, kind="ExternalInput").ap()

    y_out = nc.dram_tensor("y_out", [BPC, L, D], F32, kind="ExternalOutput").ap()
    w0_out = nc.dram_tensor("w0_out", [BPC, DH, D], F32, kind="ExternalOutput").ap()
    w1_out = nc.dram_tensor("w1_out", [BPC, D, DH], F32, kind="ExternalOutput").ap()
    w2_out = nc.dram_tensor("w2_out", [BPC, DH, D], F32, kind="ExternalOutput").ap()

    with tile.TileContext(nc) as tc, ExitStack() as ctx:
        persist = ctx.enter_context(tc.tile_pool(name="persist", bufs=1))
        lin = ctx.enter_context(tc.tile_pool(name="lin", bufs=2))
        actp = ctx.enter_context(tc.tile_pool(name="actp", bufs=2))
        act4 = ctx.enter_context(tc.tile_pool(name="act4", bufs=4))
        sp = ctx.enter_context(tc.tile_pool(name="sp", bufs=2))
        ystp = ctx.enter_context(tc.tile_pool(name="ystp", bufs=2))
        smallp = ctx.enter_context(tc.tile_pool(name="smallp", bufs=2))
        scrp = ctx.enter_context(tc.tile_pool(name="scrp", bufs=1))
        mmps = ctx.enter_context(tc.tile_pool(name="mmps", bufs=6, space="PSUM"))
        dwps = ctx.enter_context(tc.tile_pool(name="dwps", bufs=2, space="PSUM"))

        identity = persist.tile([P, P], F32, tag="identity", name="identity")
        make_identity(nc, identity)

        st = []
        for b in range(BPC):
            s = {}
            s["w0n"] = persist.tile([P, DHT, D], F32, tag=f"w0n{b}", name=f"w0n{b}")
            s["w1n"] = persist.tile([P, DT, DH], F32, tag=f"w1n{b}", name=f"w1n{b}")
            s["w2n"] = persist.tile([P, DHT, D], F32, tag=f"w2n{b}", name=f"w2n{b}")
            s["w0t"] = persist.tile([P, DT, DH], BF16, tag=f"w0t{b}", name=f"w0t{b}")
            s["w1t"] = persist.tile([P, DHT, D], BF16, tag=f"w1t{b}", name=f"w1t{b}")
            s["w2t"] = persist.tile([P, DT, DH], BF16, tag=f"w2t{b}", name=f"w2t{b}")
            s["w1nb"] = persist.tile([P, DT, DH], BF16, tag=f"w1nb{b}", name=f"w1nb{b}")
            s["sq0"] = persist.tile([P, DHT], F32, tag=f"sq0{b}", name=f"sq0{b}")
            s["sq1"] = persist.tile([P, DT], F32, tag=f"sq1{b}", name=f"sq1{b}")
            s["sq2"] = persist.tile([P, DHT], F32, tag=f"sq2{b}", name=f"sq2{b}")
            s["scr"] = scrp.tile([P, DH], F32, tag=f"scr{b}", name=f"scr{b}")
            st.append(s)

        def row_sumsq(b, wtile, nm, fd, dst):
            s = st[b]
            for m in range(nm):
                nc.scalar.activation(
                    out=s["scr"][:, :fd],
                    in_=wtile[:, m, :],
                    func=AF.Square,
                    accum_out=dst[:, m : m + 1],
                )

        def retranspose(b):
            """Regenerate bf16 transposed weight copies from fp32 masters."""
            s = st[b]
            for i in range(DT):
                ps = mmps.tile([P, DH], F32, tag="mm", name="ps_w0t")
                for j in range(DHT):
                    nc.tensor.transpose(
                        ps[:, j * P : (j + 1) * P],
                        s["w0n"][:, j, i * P : (i + 1) * P],
                        identity,
                    )
                nc.vector.tensor_copy(s["w0t"][:, i, :], ps)
            for i in range(DT):
                ps = mmps.tile([P, DH], F32, tag="mm", name="ps_w2t")
                for j in range(DHT):
                    nc.tensor.transpose(
                        ps[:, j * P : (j + 1) * P],
                        s["w2n"][:, j, i * P : (i + 1) * P],
                        identity,
                    )
                nc.vector.tensor_copy(s["w2t"][:, i, :], ps)
            for i in range(DHT):
                ps = mmps.tile([P, D], F32, tag="mm", name="ps_w1t")
                for j in range(DT):
                    nc.tensor.transpose(
                        ps[:, j * P : (j + 1) * P],
                        s["w1n"][:, j, i * P : (i + 1) * P],
                        identity,
                    )
                nc.scalar.copy(s["w1t"][:, i, :], ps)
            for m in range(DT):
                nc.scalar.copy(s["w1nb"][:, m, :], s["w1n"][:, m, :])

        def forward_half(b, qiT_h, tok0):
            """SwiGLU forward on 512 tokens; writes y_out[b, tok0:tok0+512]."""
            s = st[b]
            s_h = sp.tile([P, DHT, MB], BF16, tag=f"s{b}", name="s_h")
            for m in range(DHT):
                qg = mmps.tile([P, MB], F32, tag="mm", name="qg")
                for kk in range(DT):
                    nc.tensor.matmul(
                        qg,
                        lhsT=s["w0t"][:, kk, m * P : (m + 1) * P],
                        rhs=qiT_h[:, kk, :],
                        start=(kk == 0),
                        stop=(kk == DT - 1),
                    )
                qh = mmps.tile([P, MB], F32, tag="mm", name="qh")
                for kk in range(DT):
                    nc.tensor.matmul(
                        qh,
                        lhsT=s["w2t"][:, kk, m * P : (m + 1) * P],
                        rhs=qiT_h[:, kk, :],
                        start=(kk == 0),
                        stop=(kk == DT - 1),
                    )
                sgf = actp.tile([P, MB], BF16, tag=f"sg{b}", name="sgf")
                nc.scalar.activation(out=sgf, in_=qg, func=AF.Silu)
                nc.vector.tensor_mul(s_h[:, m, :], sgf, qh)
            ys = ystp.tile([P, MBT, D], F32, tag=f"ys{b}", name="ys")
            for mt in range(MBT):
                yp = mmps.tile([P, D], F32, tag="mm", name="yp")
                for kk in range(DHT):
                    nc.tensor.matmul(
                        yp,
                        lhsT=s_h[:, kk, mt * P : (mt + 1) * P],
                        rhs=s["w1t"][:, kk, :],
                        start=(kk == 0),
                        stop=(kk == DHT - 1),
                    )
                nc.scalar.copy(ys[:, mt, :], yp)
            nc.sync.dma_start(
                out=y_out[b, tok0 : tok0 + MB].rearrange("(t p) d -> p t d", p=P),
                in_=ys[:],
            )

        def load_qiT(b, tok0):
            qiT_h = lin.tile([P, DT, MB], BF16, tag=f"qiT{b}", name="qiT_h")
            for i in range(DT):
                nc.sync.dma_start_transpose(
                    qiT_h[:, i, :], qb[b, tok0 : tok0 + MB, i * P : (i + 1) * P]
                )
            return qiT_h

        def renorm(b, wtile, nm, fd, sqtile):
            s = st[b]
            ssn = smallp.tile([P, nm], F32, tag=f"ssn{b}", name="ssn")
            row_sumsq(b, wtile, nm, fd, ssn)
            rr = smallp.tile([P, nm], F32, tag=f"rr{b}", name="rr")
            nc.vector.reciprocal(rr, ssn)
            fac = smallp.tile([P, nm], F32, tag=f"fac{b}", name="fac")
            nc.vector.tensor_mul(fac, rr, sqtile)
            nc.scalar.activation(out=fac, in_=fac, func=AF.Sqrt)
            for m in range(nm):
                nc.vector.tensor_scalar_mul(
                    wtile[:, m, :], wtile[:, m, :], fac[:, m : m + 1]
                )

        def update(b, c):
            s = st[b]
            kiT = lin.tile([P, DT, MB], BF16, tag=f"kiT{b}", name="kiT")
            viT = lin.tile([P, DT, MB], BF16, tag=f"viT{b}", name="viT")
            for i in range(DT):
                nc.sync.dma_start_transpose(
                    kiT[:, i, :], kb[b, c, :, i * P : (i + 1) * P]
                )
                nc.sync.dma_start_transpose(
                    viT[:, i, :], vb[b, c, :, i * P : (i + 1) * P]
                )
            ki = lin.tile([P, MBT, D], BF16, tag=f"ki{b}", name="ki")
            nc.sync.dma_start(out=ki[:], in_=kb[b, c].rearrange("(t p) d -> p t d", p=P))
            vi = lin.tile([P, MBT, D], BF16, tag=f"vi{b}", name="vi")
            nc.sync.dma_start(out=vi[:], in_=vb[b, c].rearrange("(t p) d -> p t d", p=P))
            l0t = lin.tile([P, MBT, D], BF16, tag=f"l0t{b}", name="l0t")
            nc.sync.dma_start(out=l0t[:], in_=l0[b, c].rearrange("(t p) d -> p t d", p=P))
            l2t = lin.tile([P, MBT, D], BF16, tag=f"l2t{b}", name="l2t")
            nc.sync.dma_start(out=l2t[:], in_=l2[b, c].rearrange("(t p) d -> p t d", p=P))
            l1t = lin.tile([P, MBT, DH], BF16, tag=f"l1t{b}", name="l1t")
            nc.sync.dma_start(out=l1t[:], in_=l1[b, c].rearrange("(t p) h -> p t h", p=P))

            a1s, dgbs, dhids, kl0s, kl2s = [], [], [], [], []
            for t in range(MBT):
                g = mmps.tile([P, DH], F32, tag="mm", name="g")
                for kk in range(DT):
                    nc.tensor.matmul(
                        g,
                        lhsT=kiT[:, kk, t * P : (t + 1) * P],
                        rhs=s["w0t"][:, kk, :],
                        start=(kk == 0),
                        stop=(kk == DT - 1),
                    )
                h = mmps.tile([P, DH], F32, tag="mm", name="h")
                for kk in range(DT):
                    nc.tensor.matmul(
                        h,
                        lhsT=kiT[:, kk, t * P : (t + 1) * P],
                        rhs=s["w2t"][:, kk, :],
                        start=(kk == 0),
                        stop=(kk == DT - 1),
                    )
                dhd = mmps.tile([P, DH], F32, tag="mm", name="dhd")
                for kk in range(DT):
                    nc.tensor.matmul(
                        dhd,
                        lhsT=viT[:, kk, t * P : (t + 1) * P],
                        rhs=s["w1nb"][:, kk, :],
                        start=(kk == 0),
                        stop=(kk == DT - 1),
                    )
                sg = actp.tile([P, DH], BF16, tag=f"sg{b}", name="sg")
                nc.scalar.activation(out=sg, in_=g, func=AF.Silu)
                ds_ = actp.tile([P, DH], BF16, tag=f"ds{b}", name="ds_")
                nc.scalar.activation(out=ds_, in_=g, func=AF.Derivative_silu)
                hs = actp.tile([P, DH], BF16, tag=f"hs{b}", name="hs")
                nc.scalar.copy(hs, h)
                dhs = actp.tile([P, DH], BF16, tag=f"dhs{b}", name="dhs")
                nc.scalar.copy(dhs, dhd)
                t1 = actp.tile([P, DH], BF16, tag=f"tmp{b}", name="t1")
                nc.vector.tensor_mul(t1, sg, hs)
                a1 = act4.tile([P, DH], BF16, tag=f"a1{b}", name="a1")
                nc.vector.tensor_mul(a1, t1, l1t[:, t, :])
                dhid = act4.tile([P, DH], BF16, tag=f"dhid{b}", name="dhid")
                nc.gpsimd.tensor_mul(dhid, dhs, sg)
                dga = actp.tile([P, DH], BF16, tag=f"tmp{b}", name="dga")
                nc.gpsimd.tensor_mul(dga, dhs, hs)
                dgb = act4.tile([P, DH], BF16, tag=f"dgb{b}", name="dgb")
                nc.vector.tensor_mul(dgb, dga, ds_)
                kl0 = act4.tile([P, D], BF16, tag=f"kl0{b}", name="kl0")
                nc.vector.tensor_mul(kl0, ki[:, t, :], l0t[:, t, :])
                kl2 = act4.tile([P, D], BF16, tag=f"kl2{b}", name="kl2")
                nc.gpsimd.tensor_mul(kl2, ki[:, t, :], l2t[:, t, :])
                a1s.append(a1)
                dgbs.append(dgb)
                dhids.append(dhid)
                kl0s.append(kl0)
                kl2s.append(kl2)

            # dw0 = dgb.T @ kl0  -> [DH, D] natural; two m-tiles per psum bank
            for mp in range(DHT // 2):
                ps = dwps.tile([P, 2 * D], F32, tag="dw", name="ps_dw0")
                for mh in range(2):
                    m = 2 * mp + mh
                    for t in range(MBT):
                        nc.tensor.matmul(
                            ps[:, mh * D : (mh + 1) * D],
                            lhsT=dgbs[t][:, m * P : (m + 1) * P],
                            rhs=kl0s[t],
                            start=(t == 0),
                            stop=(t == MBT - 1),
                        )
                nc.vector.tensor_add(
                    s["w0n"][:, 2 * mp : 2 * mp + 2, :],
                    s["w0n"][:, 2 * mp : 2 * mp + 2, :],
                    ps.rearrange("p (m d) -> p m d", m=2),
                )
            # dw2 = dhid.T @ kl2 -> [DH, D] natural
            for mp in range(DHT // 2):
                ps = dwps.tile([P, 2 * D], F32, tag="dw", name="ps_dw2")
                for mh in range(2):
                    m = 2 * mp + mh
                    for t in range(MBT):
                        nc.tensor.matmul(
                            ps[:, mh * D : (mh + 1) * D],
                            lhsT=dhids[t][:, m * P : (m + 1) * P],
                            rhs=kl2s[t],
                            start=(t == 0),
                            stop=(t == MBT - 1),
                        )
                nc.vector.tensor_add(
                    s["w2n"][:, 2 * mp : 2 * mp + 2, :],
                    s["w2n"][:, 2 * mp : 2 * mp + 2, :],
                    ps.rearrange("p (m d) -> p m d", m=2),
                )
            # dw1 = vi.T @ a1 -> [D, DH] natural
            for m in range(DT):
                ps = dwps.tile([P, DH], F32, tag="dw", name="ps_dw1")
                for t in range(MBT):
                    nc.tensor.matmul(
                        ps,
                        lhsT=vi[:, t, m * P : (m + 1) * P],
                        rhs=a1s[t],
                        start=(t == 0),
                        stop=(t == MBT - 1),
                    )
                nc.vector.tensor_add(s["w1n"][:, m, :], s["w1n"][:, m, :], ps)

            renorm(b, s["w0n"], DHT, D, s["sq0"])
            renorm(b, s["w1n"], DT, DH, s["sq1"])
            renorm(b, s["w2n"], DHT, D, s["sq2"])
            retranspose(b)

        # ---- setup ----
        for b in range(BPC):
            s = st[b]
            nc.sync.dma_start(out=s["w0n"][:], in_=w0_in[b].rearrange("(m p) d -> p m d", p=P))
            nc.sync.dma_start(out=s["w1n"][:], in_=w1_in[b].rearrange("(m p) h -> p m h", p=P))
            nc.sync.dma_start(out=s["w2n"][:], in_=w2_in[b].rearrange("(m p) d -> p m d", p=P))
            row_sumsq(b, s["w0n"], DHT, D, s["sq0"])
            row_sumsq(b, s["w1n"], DT, DH, s["sq1"])
            row_sumsq(b, s["w2n"], DHT, D, s["sq2"])
            retranspose(b)

        # out_first: forward on tokens [0, FIRST) with initial weights
        for b in range(BPC):
            qfT = load_qiT(b, 0)
            forward_half(b, qfT, 0)

        # ---- chunked recurrence ----
        for c in range(NCH):
            for b in range(BPC):
                update(b, c)
            for b in range(BPC):
                base = FIRST + c * UE
                for u in range(2):
                    qiT_h = load_qiT(b, base + u * MB)
                    forward_half(b, qiT_h, base + u * MB)

        # ---- final weights out ----
        for b in range(BPC):
            s = st[b]
            nc.sync.dma_start(out=w0_out[b].rearrange("(m p) d -> p m d", p=P), in_=s["w0n"][:])
            nc.sync.dma_start(out=w1_out[b].rearrange("(m p) h -> p m h", p=P), in_=s["w1n"][:])
            nc.sync.dma_start(out=w2_out[b].rearrange("(m p) d -> p m d", p=P), in_=s["w2n"][:])

    nc.compile()
    return nc


_CACHED_NC = None


def _get_nc():
    global _CACHED_NC
    if _CACHED_NC is None:
        _CACHED_NC = build_program()
    return _CACHED_NC


def make_in_maps(w0, w1, w2, q, k, v, lr0, lr1, lr2):
    w0 = np.ascontiguousarray(np.asarray(w0, dtype=np.float32))
    w1 = np.ascontiguousarray(np.asarray(w1, dtype=np.float32))
    w2 = np.ascontiguousarray(np.asarray(w2, dtype=np.float32))
    q = np.asarray(q, dtype=np.float32)
    k = np.asarray(k, dtype=np.float32)
    v = np.asarray(v, dtype=np.float32)
    lr0 = np.asarray(lr0, dtype=np.float32)
    lr1 = np.asarray(lr1, dtype=np.float32)
    lr2 = np.asarray(lr2, dtype=np.float32)

    idx = FIRST + np.arange(NCH)[:, None] * UE + np.arange(MB)[None, :]  # [NCH, MB]
    qb = q.astype(BF_NP)
    kbm = k[:, idx].astype(BF_NP)
    vbm = v[:, idx].astype(BF_NP)
    l0m = (lr0[:, idx] * W_SCALE).astype(BF_NP)
    l1m = (lr1[:, idx] * W_SCALE).astype(BF_NP)
    l2m = (lr2[:, idx] * W_SCALE).astype(BF_NP)

    in_maps = []
    for core in range(NCORES):
        sl = slice(BPC * core, BPC * (core + 1))
        in_maps.append(
            {
                "w0_in": w0[sl],
                "w1_in": w1[sl],
                "w2_in": w2[sl],
                "qb": np.ascontiguousarray(qb[sl]),
                "kb": np.ascontiguousarray(kbm[sl]),
                "vb": np.ascontiguousarray(vbm[sl]),
                "l0": np.ascontiguousarray(l0m[sl]),
                "l1": np.ascontiguousarray(l1m[sl]),
                "l2": np.ascontiguousarray(l2m[sl]),
            }
        )
    return in_maps


def run_in_maps(in_maps):
    nc = _get_nc()
    res = run_bass_kernel_spmd(nc, in_maps, core_ids=list(range(NCORES)))
    return res


def kernel(w0, w1, w2, q, k, v, lr0, lr1, lr2):
    in_maps = make_in_maps(w0, w1, w2, q, k, v, lr0, lr1, lr2)
    res = run_in_maps(in_maps)
    y = np.concatenate([r["y_out"] for r in res.results], axis=0)
    w0f = np.concatenate([r["w0_out"] for r in res.results], axis=0)
    w1f = np.concatenate([r["w1_out"] for r in res.results], axis=0)
    w2f = np.concatenate([r["w2_out"] for r in res.results], axis=0)
    return y, w0f, w1f, w2f


# revision 5
# speedup vs baseline: 24.7552x; 24.7552x over previous
"""Trainium2 Bass kernel for nn_ARFastWeightSwiGLU.

Auto-regressive fast-weight SwiGLU: chunked recurrence over 4 chunks.
Per chunk: a rank-MB weight update (forward + backward-style products on a
512-token mini-batch, per-row renormalization) followed by a SwiGLU forward
over the chunk's 1024 query tokens with the updated weights.

Sharding: pure data/head parallel over the merged batch*head dim b (16),
2 b-slices per NeuronCore, 8 cores, no collectives.

Layout strategy (per b):
  - master weights fp32 natural layout in SBUF, updated + renormalized there
  - bf16 transposed copies (regenerated per chunk via PE transpose) feed all
    matmuls; activations bf16; PSUM accumulation fp32
  - k/v/q arrive pre-cast to bf16 from the host; transposed operand tiles are
    loaded straight from HBM with the XBAR DMA transpose (bf16-only path)
  - lr tensors arrive pre-scaled by w_scale, so the update products need no
    extra scaling pass
"""

import sys

sys.path.insert(0, "/opt/trn_rl_repo")

from contextlib import ExitStack

import ml_dtypes
import numpy as np

import concourse.bacc as bacc
import concourse.bass as bass
import concourse.mybir as mybir
import concourse.tile as tile
from concourse.bass_utils import run_bass_kernel_spmd
from concourse.masks import make_identity

# Problem constants (hardcoded; kernel.py must be self-contained)
B, L, D, DH = 16, 4608, 256, 512
MB, UE = 512, 1024
W_SCALE = 0.01
FIRST = UE - MB          # 512
NCH = (L - FIRST) // UE  # 4
NCORES = 8
BPC = B // NCORES        # 2 b-slices per core
P = 128
MBT = MB // P            # 4 token tiles per mini-batch
DHT = DH // P            # 4 partition tiles of the hidden dim
DT = D // P              # 2 partition tiles of the head dim

F32 = mybir.dt.float32
BF16 = mybir.dt.bfloat16
AF = mybir.ActivationFunctionType

BF_NP = ml_dtypes.bfloat16


def build_program():
    nc = bacc.Bacc("TRN2", target_bir_lowering=False, debug=False)

    w0_in = nc.dram_tensor("w0_in", [BPC, DH, D], F32, kind="ExternalInput").ap()
    w1_in = nc.dram_tensor("w1_in", [BPC, D, DH], F32, kind="ExternalInput").ap()
    w2_in = nc.dram_tensor("w2_in", [BPC, DH, D], F32, kind="ExternalInput").ap()
    qb = nc.dram_tensor("qb", [BPC, L, D], BF16, kind="ExternalInput").ap()
    kb = nc.dram_tensor("kb", [BPC, NCH, MB, D], BF16, kind="ExternalInput").ap()
    vb = nc.dram_tensor("vb", [BPC, NCH, MB, D], BF16, kind="ExternalInput").ap()
    l0 = nc.dram_tensor("l0", [BPC, NCH, MB, D], BF16, kind="ExternalInput").ap()
    l1 = nc.dram_tensor("l1", [BPC, NCH, MB, DH], BF16, kind="ExternalInput").ap()
    l2 = nc.dram_tensor("l2", [BPC, NCH, MB, D], BF16, kind="ExternalInput").ap()

    y_out = nc.dram_tensor("y_out", [BPC, L, D], F32, kind="ExternalOutput").ap()
    w0_out = nc.dram_tensor("w0_out", [BPC, DH, D], F32, kind="ExternalOutput").ap()
    w1_out = nc.dram_tensor("w1_out", [BPC, D, DH], F32, kind="ExternalOutput").ap()
    w2_out = nc.dram_tensor("w2_out", [BPC, DH, D], F32, kind="ExternalOutput").ap()

    with tile.TileContext(nc) as tc, ExitStack() as ctx:
        persist = ctx.enter_context(tc.tile_pool(name="persist", bufs=1))
        lin = ctx.enter_context(tc.tile_pool(name="lin", bufs=2))
        actp = ctx.enter_context(tc.tile_pool(name="actp", bufs=2))
        act4 = ctx.enter_context(tc.tile_pool(name="act4", bufs=4))
        sp = ctx.enter_context(tc.tile_pool(name="sp", bufs=2))
        ystp = ctx.enter_context(tc.tile_pool(name="ystp", bufs=2))
        smallp = ctx.enter_context(tc.tile_pool(name="smallp", bufs=2))
        scrp = ctx.enter_context(tc.tile_pool(name="scrp", bufs=1))
        mmps = ctx.enter_context(tc.tile_pool(name="mmps", bufs=6, space="PSUM"))
        dwps = ctx.enter_context(tc.tile_pool(name="dwps", bufs=2, space="PSUM"))

        identity = persist.tile([P, P], F32, tag="identity", name="identity")
        make_identity(nc, identity)

        st = []
        for b in range(BPC):
            s = {}
            s["w0n"] = persist.tile([P, DHT, D], F32, tag=f"w0n{b}", name=f"w0n{b}")
            s["w1n"] = persist.tile([P, DT, DH], F32, tag=f"w1n{b}", name=f"w1n{b}")
            s["w2n"] = persist.tile([P, DHT, D], F32, tag=f"w2n{b}", name=f"w2n{b}")
            s["w0t"] = persist.tile([P, DT, DH], BF16, tag=f"w0t{b}", name=f"w0t{b}")
            s["w1t"] = persist.tile([P, DHT, D], BF16, tag=f"w1t{b}", name=f"w1t{b}")
            s["w2t"] = persist.tile([P, DT, DH], BF16, tag=f"w2t{b}", name=f"w2t{b}")
            s["w1nb"] = persist.tile([P, DT, DH], BF16, tag=f"w1nb{b}", name=f"w1nb{b}")
            s["sq0"] = persist.tile([P, DHT], F32, tag=f"sq0{b}", name=f"sq0{b}")
            s["sq1"] = persist.tile([P, DT], F32, tag=f"sq1{b}", name=f"sq1{b}")
            s["sq2"] = persist.tile([P, DHT], F32, tag=f"sq2{b}", name=f"sq2{b}")
            s["scr"] = scrp.tile([P, DH], F32, tag=f"scr{b}", name=f"scr{b}")
            st.append(s)

        def row_sumsq(b, wtile, nm, fd, dst):
            s = st[b]
            for m in range(nm):
                nc.scalar.activation(
                    out=s["scr"][:, :fd],
                    in_=wtile[:, m, :],
                    func=AF.Square,
                    accum_out=dst[:, m : m + 1],
                )

        def retranspose(b):
            """Regenerate bf16 transposed weight copies from fp32 masters."""
            s = st[b]
            for i in range(DT):
                ps = mmps.tile([P, DH], F32, tag="mm", name="ps_w0t")
                for j in range(DHT):
                    nc.tensor.transpose(
                        ps[:, j * P : (j + 1) * P],
                        s["w0n"][:, j, i * P : (i + 1) * P],
                        identity,
                    )
                nc.vector.tensor_copy(s["w0t"][:, i, :], ps)
            for i in range(DT):
                ps = mmps.tile([P, DH], F32, tag="mm", name="ps_w2t")
                for j in range(DHT):
                    nc.tensor.transpose(
                        ps[:, j * P : (j + 1) * P],
                        s["w2n"][:, j, i * P : (i + 1) * P],
                        identity,
                    )
                nc.vector.tensor_copy(s["w2t"][:, i, :], ps)
            for i in range(DHT):
                ps = mmps.tile([P, D], F32, tag="mm", name="ps_w1t")
                for j in range(DT):
                    nc.tensor.transpose(
                        ps[:, j * P : (j + 1) * P],
                        s["w1n"][:, j, i * P : (i + 1) * P],
                        identity,
                    )
                nc.scalar.copy(s["w1t"][:, i, :], ps)
            for m in range(DT):
                nc.scalar.copy(s["w1nb"][:, m, :], s["w1n"][:, m, :])

        def forward_half(b, qiT_h, tok0):
            """SwiGLU forward on 512 tokens; writes y_out[b, tok0:tok0+512]."""
            s = st[b]
            s_h = sp.tile([P, DHT, MB], BF16, tag=f"s{b}", name="s_h")
            for m in range(DHT):
                qg = mmps.tile([P, MB], F32, tag="mm", name="qg")
                for kk in range(DT):
                    nc.tensor.matmul(
                        qg,
                        lhsT=s["w0t"][:, kk, m * P : (m + 1) * P],
                        rhs=qiT_h[:, kk, :],
                        start=(kk == 0),
                        stop=(kk == DT - 1),
                    )
                qh = mmps.tile([P, MB], F32, tag="mm", name="qh")
                for kk in range(DT):
                    nc.tensor.matmul(
                        qh,
                        lhsT=s["w2t"][:, kk, m * P : (m + 1) * P],
                        rhs=qiT_h[:, kk, :],
                        start=(kk == 0),
                        stop=(kk == DT - 1),
                    )
                sgf = actp.tile([P, MB], BF16, tag=f"sg{b}", name="sgf")
                nc.scalar.activation(out=sgf, in_=qg, func=AF.Silu)
                nc.vector.tensor_mul(s_h[:, m, :], sgf, qh)
            ys = ystp.tile([P, MBT, D], F32, tag=f"ys{b}", name="ys")
            for mt in range(MBT):
                yp = mmps.tile([P, D], F32, tag="mm", name="yp")
                for kk in range(DHT):
                    nc.tensor.matmul(
                        yp,
                        lhsT=s_h[:, kk, mt * P : (mt + 1) * P],
                        rhs=s["w1t"][:, kk, :],
                        start=(kk == 0),
                        stop=(kk == DHT - 1),
                    )
                nc.scalar.copy(ys[:, mt, :], yp)
            nc.sync.dma_start(
                out=y_out[b, tok0 : tok0 + MB].rearrange("(t p) d -> p t d", p=P),
                in_=ys[:],
            )

        def load_qiT(b, tok0):
            qiT_h = lin.tile([P, DT, MB], BF16, tag=f"qiT{b}", name="qiT_h")
            for i in range(DT):
                nc.sync.dma_start_transpose(
                    qiT_h[:, i, :], qb[b, tok0 : tok0 + MB, i * P : (i + 1) * P]
                )
            return qiT_h

        def renorm(b, wtile, nm, fd, sqtile):
            s = st[b]
            ssn = smallp.tile([P, nm], F32, tag=f"ssn{b}", name="ssn")
            row_sumsq(b, wtile, nm, fd, ssn)
            rr = smallp.tile([P, nm], F32, tag=f"rr{b}", name="rr")
            nc.vector.reciprocal(rr, ssn)
            fac = smallp.tile([P, nm], F32, tag=f"fac{b}", name="fac")
            nc.vector.tensor_mul(fac, rr, sqtile)
            nc.scalar.activation(out=fac, in_=fac, func=AF.Sqrt)
            for m in range(nm):
                nc.vector.tensor_scalar_mul(
                    wtile[:, m, :], wtile[:, m, :], fac[:, m : m + 1]
                )

        def update(b, c):
            s = st[b]
            kiT = lin.tile([P, DT, MB], BF16, tag=f"kiT{b}", name="kiT")
            viT = lin.tile([P, DT, MB], BF16, tag=f"viT{b}", name="viT")
            for i in range(DT):
                nc.sync.dma_start_transpose(
                    kiT[:, i, :], kb[b, c, :, i * P : (i + 1) * P]
                )
                nc.sync.dma_start_transpose(
                    viT[:, i, :], vb[b, c, :, i * P : (i + 1) * P]
                )
            ki = lin.tile([P, MBT, D], BF16, tag=f"ki{b}", name="ki")
            nc.sync.dma_start(out=ki[:], in_=kb[b, c].rearrange("(t p) d -> p t d", p=P))
            vi = lin.tile([P, MBT, D], BF16, tag=f"vi{b}", name="vi")
            nc.sync.dma_start(out=vi[:], in_=vb[b, c].rearrange("(t p) d -> p t d", p=P))
            l0t = lin.tile([P, MBT, D], BF16, tag=f"l0t{b}", name="l0t")
            nc.sync.dma_start(out=l0t[:], in_=l0[b, c].rearrange("(t p) d -> p t d", p=P))
            l2t = lin.tile([P, MBT, D], BF16, tag=f"l2t{b}", name="l2t")
            nc.sync.dma_start(out=l2t[:], in_=l2[b, c].rearrange("(t p) d -> p t d", p=P))
            l1t = lin.tile([P, MBT, DH], BF16, tag=f"l1t{b}", name="l1t")
            nc.sync.dma_start(out=l1t[:], in_=l1[b, c].rearrange("(t p) h -> p t h", p=P))

            a1s, dgbs, dhids, kl0s, kl2s = [], [], [], [], []
            for t in range(MBT):
                g = mmps.tile([P, DH], F32, tag="mm", name="g")
                for kk in range(DT):
                    nc.tensor.matmul(
                        g,
                        lhsT=kiT[:, kk, t * P : (t + 1) * P],
                        rhs=s["w0t"][:, kk, :],
                        start=(kk == 0),
                        stop=(kk == DT - 1),
                    )
                h = mmps.tile([P, DH], F32, tag="mm", name="h")
                for kk in range(DT):
                    nc.tensor.matmul(
                        h,
                        lhsT=kiT[:, kk, t * P : (t + 1) * P],
                        rhs=s["w2t"][:, kk, :],
                        start=(kk == 0),
                        stop=(kk == DT - 1),
                    )
                dhd = mmps.tile([P, DH], F32, tag="mm", name="dhd")
                for kk in range(DT):
                    nc.tensor.matmul(
                        dhd,
                        lhsT=viT[:, kk, t * P : (t + 1) * P],
                        rhs=s["w1nb"][:, kk, :],
                        start=(kk == 0),
                        stop=(kk == DT - 1),
                    )
                sg = actp.tile([P, DH], BF16, tag=f"sg{b}", name="sg")
                nc.scalar.activation(out=sg, in_=g, func=AF.Silu)
                ds_ = actp.tile([P, DH], BF16, tag=f"ds{b}", name="ds_")
                nc.scalar.activation(out=ds_, in_=g, func=AF.Derivative_silu)
                hs = actp.tile([P, DH], BF16, tag=f"hs{b}", name="hs")
                nc.scalar.copy(hs, h)
                dhs = actp.tile([P, DH], BF16, tag=f"dhs{b}", name="dhs")
                nc.scalar.copy(dhs, dhd)
                t1 = actp.tile([P, DH], BF16, tag=f"tmp{b}", name="t1")
                nc.vector.tensor_mul(t1, sg, hs)
                a1 = act4.tile([P, DH], BF16, tag=f"a1{b}", name="a1")
                nc.vector.tensor_mul(a1, t1, l1t[:, t, :])
                dhid = act4.tile([P, DH], BF16, tag=f"dhid{b}", name="dhid")
                nc.gpsimd.tensor_mul(dhid, dhs, sg)
                dga = actp.tile([P, DH], BF16, tag=f"tmp{b}", name="dga")
                nc.gpsimd.tensor_mul(dga, dhs, hs)
                dgb = act4.tile([P, DH], BF16, tag=f"dgb{b}", name="dgb")
                nc.vector.tensor_mul(dgb, dga, ds_)
                kl0 = act4.tile([P, D], BF16, tag=f"kl0{b}", name="kl0")
                nc.vector.tensor_mul(kl0, ki[:, t, :], l0t[:, t, :])
                kl2 = act4.tile([P, D], BF16, tag=f"kl2{b}", name="kl2")
                nc.gpsimd.tensor_mul(kl2, ki[:, t, :], l2t[:, t, :])
                a1s.append(a1)
                dgbs.append(dgb)
                dhids.append(dhid)
                kl0s.append(kl0)
                kl2s.append(kl2)

            # dw0 = dgb.T @ kl0  -> [DH, D] natural; two m-tiles per psum bank
            for mp in range(DHT // 2):
                ps = dwps.tile([P, 2 * D], F32, tag="dw", name="ps_dw0")
                for mh in range(2):
                    m = 2 * mp + mh
                    for t in range(MBT):
                        nc.tensor.matmul(
                            ps[:, mh * D : (mh + 1) * D],
                            lhsT=dgbs[t][:, m * P : (m + 1) * P],
                            rhs=kl0s[t],
                            start=(t == 0),
                            stop=(t == MBT - 1),
                        )
                nc.vector.tensor_add(
                    s["w0n"][:, 2 * mp : 2 * mp + 2, :],
                    s["w0n"][:, 2 * mp : 2 * mp + 2, :],
                    ps.rearrange("p (m d) -> p m d", m=2),
                )
            # dw2 = dhid.T @ kl2 -> [DH, D] natural
            for mp in range(DHT // 2):
                ps = dwps.tile([P, 2 * D], F32, tag="dw", name="ps_dw2")
                for mh in range(2):
                    m = 2 * mp + mh
                    for t in range(MBT):
                        nc.tensor.matmul(
                            ps[:, mh * D : (mh + 1) * D],
                            lhsT=dhids[t][:, m * P : (m + 1) * P],
                            rhs=kl2s[t],
                            start=(t == 0),
                            stop=(t == MBT - 1),
                        )
                nc.vector.tensor_add(
                    s["w2n"][:, 2 * mp : 2 * mp + 2, :],
                    s["w2n"][:, 2 * mp : 2 * mp + 2, :],
                    ps.rearrange("p (m d) -> p m d", m=2),
                )
            # dw1 = vi.T @ a1 -> [D, DH] natural
            for m in range(DT):
                ps = dwps.tile([P, DH], F32, tag="dw", name="ps_dw1")
                for t in range(MBT):
                    nc.tensor.matmul(
                        ps,
                        lhsT=vi[:, t, m * P : (m + 1) * P],
                        rhs=a1s[t],
                        start=(t == 0),
                        stop=(t == MBT - 1),
                    )
                nc.vector.tensor_add(s["w1n"][:, m, :], s["w1n"][:, m, :], ps)

            renorm(b, s["w0n"], DHT, D, s["sq0"])
            renorm(b, s["w1n"], DT, DH, s["sq1"])
            renorm(b, s["w2n"], DHT, D, s["sq2"])
            retranspose(b)

        # ---- setup ----
        for b in range(BPC):
            s = st[b]
            nc.sync.dma_start(out=s["w0n"][:], in_=w0_in[b].rearrange("(m p) d -> p m d", p=P))
            nc.sync.dma_start(out=s["w1n"][:], in_=w1_in[b].rearrange("(m p) h -> p m h", p=P))
            nc.sync.dma_start(out=s["w2n"][:], in_=w2_in[b].rearrange("(m p) d -> p m d", p=P))
            row_sumsq(b, s["w0n"], DHT, D, s["sq0"])
            row_sumsq(b, s["w1n"], DT, DH, s["sq1"])
            row_sumsq(b, s["w2n"], DHT, D, s["sq2"])
            retranspose(b)

        # out_first: forward on tokens [0, FIRST) with initial weights
        for b in range(BPC):
            qfT = load_qiT(b, 0)
            forward_half(b, qfT, 0)

        # ---- chunked recurrence ----
        for c in range(NCH):
            for b in range(BPC):
                update(b, c)
            for b in range(BPC):
                base = FIRST + c * UE
                for u in range(2):
                    qiT_h = load_qiT(b, base + u * MB)
                    forward_half(b, qiT_h, base + u * MB)

        # ---- final weights out ----
        for b in range(BPC):
            s = st[b]
            nc.sync.dma_start(out=w0_out[b].rearrange("(m p) d -> p m d", p=P), in_=s["w0n"][:])
            nc.sync.dma_start(out=w1_out[b].rearrange("(m p) h -> p m h", p=P), in_=s["w1n"][:])
            nc.sync.dma_start(out=w2_out[b].rearrange("(m p) d -> p m d", p=P), in_=s["w2n"][:])

    nc.compile()
    return nc


_CACHED_NC = None
_CACHED_RUNNER = None


def _get_nc():
    global _CACHED_NC
    if _CACHED_NC is None:
        _CACHED_NC = build_program()
    return _CACHED_NC


def _get_runner():
    """Compile once; return (fn, n_params, in_names, out_names, out_avals, mesh).

    Mirrors bass2jax.run_bass_via_pjrt's multi-core path, but keeps the jitted
    executable so repeated runs don't re-trace/re-compile or re-transfer.
    """
    global _CACHED_RUNNER
    if _CACHED_RUNNER is not None:
        return _CACHED_RUNNER

    import jax
    from jax.sharding import Mesh, PartitionSpec
    from jax.experimental.shard_map import shard_map

    from concourse import bass2jax

    bass2jax.install_neuronx_cc_hook()
    nc = _get_nc()

    partition_name = nc.partition_id_tensor.name if nc.partition_id_tensor else None
    in_names, out_names, out_avals, zero_shapes = [], [], [], []
    for alloc in nc.m.functions[0].allocations:
        if not isinstance(alloc, mybir.MemoryLocationSet):
            continue
        name = alloc.memorylocations[0].name
        if alloc.kind == "ExternalInput":
            if name != partition_name:
                in_names.append(name)
        elif alloc.kind == "ExternalOutput":
            out_names.append(name)
            shape = tuple(alloc.tensor_shape)
            dtype = mybir.dt.np(alloc.dtype)
            out_avals.append(jax.core.ShapedArray(shape, dtype))
            zero_shapes.append((shape, dtype))
    n_params = len(in_names)
    all_in_names = in_names + out_names
    if partition_name is not None:
        all_in_names = all_in_names + [partition_name]

    def _body(*args):
        operands = list(args)
        if partition_name is not None:
            operands.append(bass2jax.partition_id_tensor())
        outs = bass2jax._bass_exec_p.bind(
            *operands,
            out_avals=tuple(out_avals),
            in_names=tuple(all_in_names),
            out_names=tuple(out_names),
            lowering_input_output_aliases=(),
            sim_require_finite=True,
            sim_require_nnan=True,
            nc=nc,
        )
        return tuple(outs)

    devices = jax.devices()[:NCORES]
    mesh = Mesh(np.asarray(devices), ("core",))
    n_outs = len(out_names)
    in_specs = (PartitionSpec("core"),) * (n_params + n_outs)
    out_specs = (PartitionSpec("core"),) * n_outs
    donate = tuple(range(n_params, n_params + n_outs))
    fn = jax.jit(
        shard_map(_body, mesh=mesh, in_specs=in_specs, out_specs=out_specs, check_rep=False),
        donate_argnums=donate,
        keep_unused=True,
    )
    _CACHED_RUNNER = (fn, n_params, in_names, out_names, out_avals, zero_shapes, mesh)
    return _CACHED_RUNNER


def make_in_maps(w0, w1, w2, q, k, v, lr0, lr1, lr2):
    w0 = np.ascontiguousarray(np.asarray(w0, dtype=np.float32))
    w1 = np.ascontiguousarray(np.asarray(w1, dtype=np.float32))
    w2 = np.ascontiguousarray(np.asarray(w2, dtype=np.float32))
    q = np.asarray(q, dtype=np.float32)
    k = np.asarray(k, dtype=np.float32)
    v = np.asarray(v, dtype=np.float32)
    lr0 = np.asarray(lr0, dtype=np.float32)
    lr1 = np.asarray(lr1, dtype=np.float32)
    lr2 = np.asarray(lr2, dtype=np.float32)

    idx = FIRST + np.arange(NCH)[:, None] * UE + np.arange(MB)[None, :]  # [NCH, MB]
    qb = q.astype(BF_NP)
    kbm = k[:, idx].astype(BF_NP)
    vbm = v[:, idx].astype(BF_NP)
    l0m = (lr0[:, idx] * W_SCALE).astype(BF_NP)
    l1m = (lr1[:, idx] * W_SCALE).astype(BF_NP)
    l2m = (lr2[:, idx] * W_SCALE).astype(BF_NP)

    in_maps = []
    for core in range(NCORES):
        sl = slice(BPC * core, BPC * (core + 1))
        in_maps.append(
            {
                "w0_in": w0[sl],
                "w1_in": w1[sl],
                "w2_in": w2[sl],
                "qb": np.ascontiguousarray(qb[sl]),
                "kb": np.ascontiguousarray(kbm[sl]),
                "vb": np.ascontiguousarray(vbm[sl]),
                "l0": np.ascontiguousarray(l0m[sl]),
                "l1": np.ascontiguousarray(l1m[sl]),
                "l2": np.ascontiguousarray(l2m[sl]),
            }
        )
    return in_maps


def _device_inputs(in_maps):
    """Concatenate per-core inputs and device_put them with the core sharding."""
    import jax
    from jax.sharding import NamedSharding, PartitionSpec

    fn, n_params, in_names, out_names, out_avals, zero_shapes, mesh = _get_runner()
    sh = NamedSharding(mesh, PartitionSpec("core"))
    concat_in = [
        np.concatenate([np.asarray(in_maps[c][name]) for c in range(NCORES)], axis=0)
        for name in in_names
    ]
    return [jax.device_put(a, sh) for a in concat_in]


def _device_zeros():
    import jax
    from jax.sharding import NamedSharding, PartitionSpec

    fn, n_params, in_names, out_names, out_avals, zero_shapes, mesh = _get_runner()
    sh = NamedSharding(mesh, PartitionSpec("core"))
    return [
        jax.device_put(np.zeros((NCORES * s[0], *s[1:]), dt), sh)
        for (s, dt) in zero_shapes
    ]


def run_device(dev_in, dev_zeros):
    fn = _get_runner()[0]
    outs = fn(*dev_in, *dev_zeros)
    return outs


def kernel(w0, w1, w2, q, k, v, lr0, lr1, lr2):
    in_maps = make_in_maps(w0, w1, w2, q, k, v, lr0, lr1, lr2)
    dev_in = _device_inputs(in_maps)
    outs = run_device(dev_in, _device_zeros())
    out_names = _get_runner()[3]
    by_name = {name: np.asarray(outs[i]) for i, name in enumerate(out_names)}
    y = by_name["y_out"].reshape(B, L, D)
    w0f = by_name["w0_out"].reshape(B, DH, D)
    w1f = by_name["w1_out"].reshape(B, D, DH)
    w2f = by_name["w2_out"].reshape(B, DH, D)
    return y, w0f, w1f, w2f
